# revision 29
# baseline (speedup 1.0000x reference)
"""Trainium2 Bass kernel for a dense causal self-attention block (RoPE + causal
softmax + QKV/O projections).

Sharding: 8 cores = 2 batches x 4 head-groups (tensor parallel over heads).
Each core computes 4 heads of attention for one batch plus the partial output
projection over its heads' dims; the host sums the 4 partial outputs per batch.

Causal-path design (fp16, head-pipelined):
  - All matmul operands are fp16 (PE streams 1 col/cycle, same as f32r, but
    halves DMA and SBUF so everything stays resident).  PSUM accumulation is
    fp32, so precision loss is one rounding per tensor hop.
  - Software pipeline over heads: [V proj || QK(h0)] -> [attn(h) || QK(h+1)]
    for h=0..2 -> [attn(h3) || O proj].  The QK projections (pure PE work)
    fill the PE while ACT/DVE chew on the previous head's softmax, so the
    scalar engine's exp never stalls the PE.
  - Scores are computed transposed: S^T[k, q] = K^T_tile.T @ Q^T.  exp runs on
    ACT straight out of PSUM in merged [128, 1024] instructions with the
    softmax shift folded into the activation bias (exp(s*scale - 5); softmax
    is shift-invariant, and -5 keeps exp sums inside fp16 range: measured
    scores/sqrt(dh) are within [-11, 9]).
  - Softmax denominator: DVE accumulates sum_ki exp-tiles into esum (fp16,
    2x-mode), then ONE GpSimd partition_all_reduce per (h, qc) produces the
    across-k denominator broadcast over partitions.  No PE matmuls for the
    denominator at all (the old design burned 1/3 of attention PE time on
    ones-vector matmuls).
  - Causality: restricted k-tile range + a 0/1 triangle mask multiply on the
    partially-valid [128, (j+1)*128] prefix of the 4 diagonal tiles per block.
  - AV^T = V_tile.T @ E^T lands in the stationary-operand layout the O
    projection wants; normalization (reciprocal_approx_fast + mul) happens at
    the AV PSUM eviction.
"""

import numpy as np

# Problem constants (hardcoded per the harness contract).
B = 2
S = 2048
D = 2048
H = 16
DH = 128
N_CORES = 8
GROUPS = 4          # head-groups (cores per batch)
HPC = H // GROUPS   # heads per core
P = 128             # SBUF partitions
QC = 512            # q/s chunk (f32 PSUM bank = 512 floats)
BIAS = -5.0         # exp(s*scale + BIAS): keeps exp sums in fp16 range
FEED = 4            # feeder matmuls emitted per attention pipeline step
LAGG = 2            # AV trails scores by LAGG groups (of G k-tiles)
G = 2               # k-tiles per exp group (one [128, 1024] ACT instruction)
N_WARM = 16         # dummy PE matmuls to ramp the clock during DMA fill

_CACHE = {}


def _ensure_paths():
    import sys
    for p in ("/opt/trn_rl_repo", "/root/.axon_site/_ro/trn_rl_repo"):
        try:
            import concourse.bass  # noqa: F401
            return
        except Exception:
            if p not in sys.path:
                sys.path.insert(0, p)
    import concourse.bass  # noqa: F401


def build_program_causal(S=S):
    """Head-pipelined fp16 causal program. Returns the compiled Bacc."""
    _ensure_paths()
    import concourse.bass as bass  # noqa: F401
    import concourse.mybir as mybir
    import concourse.tile as tile
    from concourse import bacc
    from concourse import bass_isa

    f32 = mybir.dt.float32
    f16 = mybir.dt.float16
    Exp = mybir.ActivationFunctionType.Exp
    RAdd = bass_isa.ReduceOp.add

    E = HPC * P          # per-core projection width (512)
    NDT = D // P         # d (contraction) tiles for projections
    NQC = S // QC        # q chunks
    NKT = S // P         # k tiles
    NST = S // P         # s tiles
    NOC = D // QC        # output chunks for O projection
    JB = QC // P         # diagonal blocks per q chunk (4)
    HF = P // 2
    scale = 1.0 / float(np.sqrt(DH))

    nc = bacc.Bacc("TRN2", target_bir_lowering=False, debug=False,
                   num_devices=N_CORES)

    # Pre-tiled flat DRAM layouts (built host-side): every DMA below is a
    # plain 2D contiguous-row transfer, so DIRECT2D descriptor issue on the
    # Sync engine stays cheap (the old rearranged 3D APs cost 1-6us each to
    # issue, which alone delayed the first matmul to t=15us).
    xq = nc.dram_tensor("xq", [NQC * P, NDT * QC], f16,
                        kind="ExternalInput").ap()
    wqT = nc.dram_tensor("wqT", [P, NDT * E], f16, kind="ExternalInput").ap()
    wkT = nc.dram_tensor("wkT", [P, NDT * E], f16, kind="ExternalInput").ap()
    wvT = nc.dram_tensor("wvT", [P, NDT * E], f16, kind="ExternalInput").ap()
    woT = nc.dram_tensor("woT", [P, HPC * D], f16, kind="ExternalInput").ap()
    AUXW = 2 * S + P
    aux = nc.dram_tensor("aux", [P, AUXW], f16, kind="ExternalInput").ap()
    y = nc.dram_tensor("y", [S, D], f16, kind="ExternalOutput").ap()

    xq_v = xq.rearrange("(q p) c -> p q c", p=P)

    def mm(ps, lhsT, rhs, start, stop):
        nc.tensor.matmul(ps, lhsT=lhsT, rhs=rhs, start=start, stop=stop)

    with tile.TileContext(nc) as tc:
      with tc.tile_pool(name="persist", bufs=1) as persist, \
           tc.tile_pool(name="qkp", bufs=2) as qkp, \
           tc.tile_pool(name="evp", bufs=3) as evp, \
           tc.tile_pool(name="smp", bufs=2) as smp:
        v_sb = persist.tile([P, NST * E], f16)     # V[k, e]: [p, ki*E + e]
        avt = persist.tile([P, HPC * S], f16)      # AV^T: [p=e, h*S + q]
        wo_sb = persist.tile([P, HPC * D], f16)    # [p=e, h*D + o]
        aux_sb = persist.tile([P, AUXW], f16)      # cos | sin | mask packed
        cos_sb = aux_sb[:, 0:S]
        sin_sb = aux_sb[:, S:2 * S]
        mask_sb = aux_sb[:, 2 * S:]
        et = persist.tile([P, NKT * QC], f16)      # exp scores for one (h,qc)
        bias_sb = persist.tile([P, 1], mybir.dt.float32)
        nc.vector.memset(bias_sb, BIAS)
        ones_f = persist.tile([P, P], mybir.dt.float32)
        ones16 = persist.tile([P, P], f16)
        nc.vector.memset(ones_f, 1.0)
        nc.vector.tensor_copy(ones16, ones_f)
        # (A PE warmup chain was tried here to pre-ramp the DVFS clock
        # during the DMA fill; the unavoidable data-wait gap right after it
        # resets the ramp, so it bought nothing and was removed.)

        qt_tiles = {}
        kt_tiles = {}

        def rope_evict(ps, dst, qs):
            # dst[:, qs] = ps*cos + shuffle_halves(ps)*sin_signed
            tmp = evp.tile([P, QC], f16, tag="tmp")
            nc.vector.tensor_mul(tmp[0:HF, :], ps[HF:P, :], sin_sb[0:HF, qs])
            nc.vector.tensor_mul(tmp[HF:P, :], ps[0:HF, :], sin_sb[HF:P, qs])
            ro2 = evp.tile([P, QC], f16, tag="ro2")
            nc.vector.tensor_mul(ro2, ps, cos_sb[:, qs])
            # all-f16 add runs in DVE 2x mode (~0.2us); on GpSimd it costs
            # 1.16us and queues up at stage tails, stalling the qt/kt
            # eviction -> ps_qk recycle chain (observed 0.5-1us PE gaps).
            nc.vector.tensor_add(dst[:, qs], ro2, tmp)

        class Feeder:
            def __init__(self):
                self.gens = []

            def add(self, gen):
                self.gens.append(gen)

            def prime(self, gen):
                # advance to the first yield (issues the gen's first DMA
                # now, so the data is in flight before the gen is consumed)
                try:
                    next(gen)
                except StopIteration:
                    return
                self.gens.append(gen)

            def take(self, n):
                c = 0
                while self.gens and c < n:
                    try:
                        kind = next(self.gens[0])
                    except StopIteration:
                        self.gens.pop(0)
                        continue
                    if kind == "mm":
                        c += 1

            def drain(self):
                self.take(1 << 30)

        feeder = Feeder()

        with tc.tile_pool(name="ps_qk", bufs=2, space="PSUM") as ps_qk:

          def qk_block_mms(h, x_t, qc):
              # generator: 2 x (16 matmuls + RoPE eviction) for head h, chunk qc
              qs = slice(qc * QC, (qc + 1) * QC)
              for w_sb, dst in ((wq_sb, qt_tiles[h]), (wk_sb, kt_tiles[h])):
                  ps = ps_qk.tile([P, QC], f32, tag="psqk")
                  for di in range(NDT):
                      mm(ps, w_sb[:, di * E + h * P: di * E + (h + 1) * P],
                         x_t[:, di * QC:(di + 1) * QC],
                         start=(di == 0), stop=(di == NDT - 1))
                      yield "mm"
                  rope_evict(ps, dst, qs)
                  yield "side"

          def qk_gen(h):
              # feeder stream: full QK projection for head h (streams x
              # itself, double-buffered one q-chunk ahead)
              qt_tiles[h] = qkp.tile([P, S], f16, tag="qt", name=f"qt{h}")
              kt_tiles[h] = qkp.tile([P, S], f16, tag="kt", name=f"kt{h}")
              x_cur = xp.tile([P, NDT * QC], f16, tag="x")
              nc.sync.dma_start(x_cur, xq_v[:, 0, :])
              yield "side"
              for qc in range(NQC):
                  if qc + 1 < NQC:
                      x_next = xp.tile([P, NDT * QC], f16, tag="x")
                      nc.sync.dma_start(x_next, xq_v[:, qc + 1, :])
                      yield "side"
                  else:
                      x_next = None
                  yield from qk_block_mms(h, x_cur, qc)
                  x_cur = x_next

          pending = [None]

          def finalize(on_pe=False):
              # den broadcast: partition-sum of esum replicated over
              # partitions, then reciprocal + scale at the AV eviction.
              # Runs one block late so the PE never waits on the DVE esum
              # chain.  Stages 0-3 use a GpSimd partition_all_reduce (frees
              # 512 PE cycles per block); stage 4 keeps the PE ones-matmul
              # because its result feeds the O projection immediately.
              if pending[0] is None:
                  return
              fh, fqc, f_avt, f_esum = pending[0]
              pending[0] = None
              if on_pe:
                  # stage 4: the O projection consumes avt immediately, so
                  # keep the low-latency PE ones-matmul denominator.
                  psd = ps_qk.tile([P, QC], f32, tag="psqk")
                  mm(psd, ones16, f_esum, start=True, stop=True)
                  rec = smp.tile([P, QC], f32, tag="rec", bufs=1)
                  nc.vector.reciprocal_approx_fast(out=rec, in_=psd)
                  nc.vector.tensor_mul(
                      avt[:, fh * S + fqc * QC: fh * S + (fqc + 1) * QC],
                      f_avt, rec)
              else:
                  # stages 0-3: evict the raw AV accumulator with a fast ACT
                  # copy so the PSUM bank recycles immediately, then compute
                  # the denominator on the otherwise-idle GpSimd (3.5us,
                  # but fully off the critical path) instead of burning 512
                  # PE cycles per block on a ones-matmul.
                  av_raw = smp.tile([P, QC], f16, tag="avr", bufs=2)
                  nc.scalar.copy(av_raw, f_avt)
                  den = smp.tile([P, QC], f32, tag="dnr", bufs=1)
                  nc.gpsimd.partition_all_reduce(den, f_esum, P, RAdd)
                  rec = smp.tile([P, QC], f32, tag="rec", bufs=1)
                  nc.vector.reciprocal_approx_fast(out=rec, in_=den)
                  nc.vector.tensor_mul(
                      avt[:, fh * S + fqc * QC: fh * S + (fqc + 1) * QC],
                      av_raw, rec)

          def attn_block(h, qc, ps_sc, ps_av):
              nkt = JB * (qc + 1)
              ngr = nkt // G
              qs = slice(qc * QC, (qc + 1) * QC)
              qt = qt_tiles[h]
              kt = kt_tiles[h]
              ps_avt = ps_av.tile([P, QC], f32, tag="avt")
              esum = smp.tile([P, QC], f16, tag="esum")

              def cstart(ki):
                  # first causally-valid col (within the q-chunk) of k-tile ki
                  j = ki - JB * qc
                  return j * P if j > 0 else 0

              def av_group(g):
                  for t in range(G):
                      kj = g * G + t
                      cs = cstart(kj)
                      mm(ps_avt[:, cs:],
                         v_sb[:, kj * E + h * P: kj * E + (h + 1) * P],
                         et[:, kj * QC + cs:(kj + 1) * QC],
                         start=(kj == 0), stop=(kj == nkt - 1))

              for g in range(ngr + LAGG):
                  if g < ngr:
                      ps_s = ps_sc.tile([P, G * QC], f32, tag="sc")
                      css = [cstart(g * G + t) for t in range(G)]
                      for t in range(G):
                          ki = g * G + t
                          mm(ps_s[:, t * QC + css[t]:(t + 1) * QC],
                             kt[:, ki * P:(ki + 1) * P],
                             qt[:, qc * QC + css[t]:(qc + 1) * QC],
                             start=True, stop=True)
                  if g == min(1, ngr - 1):
                      finalize()
                  feeder.take(FEED)
                  if g < ngr:
                      if not any(css):
                          nc.scalar.activation(
                              et[:, g * G * QC:(g + 1) * G * QC], ps_s, Exp,
                              scale=scale, bias=bias_sb)
                      else:
                          for t in range(G):
                              ki = g * G + t
                              nc.scalar.activation(
                                  et[:, ki * QC + css[t]:(ki + 1) * QC],
                                  ps_s[:, t * QC + css[t]:(t + 1) * QC],
                                  Exp, scale=scale, bias=bias_sb)
                      for t in range(G):
                          ki = g * G + t
                          if ki - JB * qc >= 0:
                              cs = cstart(ki)
                              # triangle block only: cols [cs, cs+P)
                              nc.vector.tensor_mul(
                                  et[:, ki * QC + cs: ki * QC + cs + P],
                                  et[:, ki * QC + cs: ki * QC + cs + P],
                                  mask_sb[:, 0:P])
                      for t in range(G):
                          ki = g * G + t
                          cs = css[t]
                          ets = et[:, ki * QC + cs:(ki + 1) * QC]
                          if ki == 0:
                              nc.vector.tensor_copy(esum, ets)
                          else:
                              nc.vector.tensor_add(
                                  esum[:, cs:], esum[:, cs:], ets)
                  if g >= LAGG:
                      av_group(g - LAGG)
              pending[0] = (h, qc, ps_avt, esum)

          # One x pool shared by stage 0 and the per-head QK feeder streams:
          # a fresh pool would be allocated in the freed SBUF region of the
          # old one, and the region-based WAR tracking would then stall the
          # feeder's first x DMA until every stage-0 read of that region
          # finished (a ~30us stall observed on hardware).  Sharing the pool
          # (and tile tag) gives seamless buffer rotation instead.
          with tc.tile_pool(name="wp", bufs=1) as wp, \
               tc.tile_pool(name="xp", bufs=3) as xp:
            wv_sb = wp.tile([P, NDT * E], f16)
            wq_sb = wp.tile([P, NDT * E], f16)
            wk_sb = wp.tile([P, NDT * E], f16)

            WG = 4

            def load_w(w_sb, w_t, g):
                nc.sync.dma_start(w_sb[:, g * WG * E:(g + 1) * WG * E],
                                  w_t[:, g * WG * E:(g + 1) * WG * E])

            # ---- stage 0: V projection || QK(h0), DMA-paced startup ----
            with tc.tile_pool(name="ps_v", bufs=4, space="PSUM") as ps_v:
                xs0 = []
                for qc in range(NQC):
                    xs0.append(xp.tile([P, NDT * QC], f16, tag="x", name=f"x0_{qc}"))
                # priming DMAs in first-use order; wv and x(qc0) interleaved
                # in di-chunks so the di-outer V loop streams at DMA pace.
                # First pair is a single di-tile so the first matmul's gate
                # is only ~256KB of transfer.
                nc.sync.dma_start(wv_sb[:, 0:E], wvT[:, 0:E])
                nc.sync.dma_start(xs0[0][:, 0:QC], xq_v[:, 0, 0:QC])
                nc.sync.dma_start(wv_sb[:, E:WG * E], wvT[:, E:WG * E])
                nc.sync.dma_start(xs0[0][:, QC:WG * QC],
                                  xq_v[:, 0, QC:WG * QC])
                for g in range(1, NDT // WG):
                    load_w(wv_sb, wvT, g)
                    nc.sync.dma_start(
                        xs0[0][:, g * WG * QC:(g + 1) * WG * QC],
                        xq_v[:, 0, g * WG * QC:(g + 1) * WG * QC])
                # wq interleaved with x1 (and wk with x2) at tile-group
                # granularity: QK(h0,qc0) needs wq at ~29us and V(qc1)
                # consumes x1 progressively — serializing either behind the
                # other's full 2MB stalls its consumer by ~2us.
                for g in range(NDT // WG):
                    load_w(wq_sb, wqT, g)
                    if NQC > 1:
                        nc.sync.dma_start(
                            xs0[1][:, g * WG * QC:(g + 1) * WG * QC],
                            xq_v[:, 1, g * WG * QC:(g + 1) * WG * QC])
                for g in range(NDT // WG):
                    load_w(wk_sb, wkT, g)
                    if NQC > 2:
                        nc.sync.dma_start(
                            xs0[2][:, g * WG * QC:(g + 1) * WG * QC],
                            xq_v[:, 2, g * WG * QC:(g + 1) * WG * QC])
                nc.sync.dma_start(aux_sb, aux)
                nc.sync.dma_start(wo_sb, woT)
                for qc in range(3, NQC):
                    nc.sync.dma_start(xs0[qc], xq_v[:, qc, :])

                qt_tiles[0] = qkp.tile([P, S], f16, tag="qt", name="qt0")
                kt_tiles[0] = qkp.tile([P, S], f16, tag="kt", name="kt0")

                def v_block(qc):
                    # di-outer so the first matmul only needs the first
                    # wv/x di-chunk; evictions on ACT (idle during stage 0).
                    psv = [ps_v.tile([P, E], f32, tag="psv", name=f"psv{sl}")
                           for sl in range(QC // P)]
                    for di in range(NDT):
                        for sl in range(QC // P):
                            mm(psv[sl],
                               xs0[qc][:, di * QC + sl * P: di * QC + (sl + 1) * P],
                               wv_sb[:, di * E:(di + 1) * E],
                               start=(di == 0), stop=(di == NDT - 1))
                    for sl in range(QC // P):
                        si = qc * (QC // P) + sl
                        if qc == NQC - 1:
                            nc.vector.tensor_copy(
                                v_sb[:, si * E:(si + 1) * E], psv[sl])
                        else:
                            nc.scalar.copy(v_sb[:, si * E:(si + 1) * E], psv[sl])

                def qk0_block(qc):
                    for _ in qk_block_mms(0, xs0[qc], qc):
                        pass

                # emission order tuned so x(qc) WAR frees line up with the
                # DMA issue order above: V0 V1 QK0 V2 QK1 V3 QK2 QK3
                order = []
                for qc in range(NQC):
                    order.append(("v", qc))
                    if qc >= 1 or NQC == 1:
                        order.append(("qk", qc - 1 if NQC > 1 else 0))
                if NQC > 1:
                    order += [("qk", NQC - 2), ("qk", NQC - 1)]
                seen = set()
                for kind, qc in order:
                    if (kind, qc) in seen:
                        continue
                    seen.add((kind, qc))
                    if kind == "v":
                        v_block(qc)
                    else:
                        qk0_block(qc)

            # ---- stages 1..3: attn(h) || QK(h+1) ----
            # Each head's QK gen is primed one stage early so its first x
            # slab is in flight before its matmuls are pulled; no drains at
            # stage boundaries (the ~16-matmul backlog of head h+1 drains
            # during head h+1's early feed steps, which only need the
            # earliest q-chunks of qt/kt).
            with tc.tile_pool(name="ps_sc", bufs=2, space="PSUM") as ps_sc, \
                 tc.tile_pool(name="ps_av", bufs=2, space="PSUM") as ps_av:
                feeder.prime(qk_gen(1))
                for h in range(HPC - 1):
                    for qc in range(NQC):
                        attn_block(h, qc, ps_sc, ps_av)
                    if h + 2 < HPC:
                        feeder.prime(qk_gen(h + 2))
                finalize()

                # ---- stage 4: attn(h3) || O projection ----
                # Shares ps_sc/ps_av: a fresh PSUM pool would reuse their
                # banks, and the region-based tracking would then stall
                # stage 4 behind the full stage-3 PSUM drain (~1us gap).
                with tc.tile_pool(name="ytp", bufs=2) as ytp:

                    def o_gen(qc):
                        for si in range(qc * (QC // P), (qc + 1) * (QC // P)):
                            yt = ytp.tile([P, D], f16, tag="yt")
                            last = si == NST - 1
                            for oc in range(NOC):
                                psy = ps_qk.tile([P, QC], f32, tag="psqk")
                                for h in range(HPC):
                                    mm(psy,
                                       avt[:, h * S + si * P:
                                           h * S + (si + 1) * P],
                                       wo_sb[:, h * D + oc * QC:
                                             h * D + (oc + 1) * QC],
                                       start=(h == 0), stop=(h == HPC - 1))
                                    yield "mm"
                                nc.scalar.copy(
                                    yt[:, oc * QC:(oc + 1) * QC], psy)
                                yield "side"
                                if last:
                                    # per-oc writes so the final transfer on
                                    # the critical tail is 128KB, not 512KB
                                    nc.sync.dma_start(
                                        y[si * P:(si + 1) * P,
                                          oc * QC:(oc + 1) * QC],
                                        yt[:, oc * QC:(oc + 1) * QC])
                                    yield "side"
                            if not last:
                                nc.sync.dma_start(
                                    y[si * P:(si + 1) * P, :], yt)
                                yield "side"

                    for qc in range(NQC):
                        attn_block(HPC - 1, qc, ps_sc, ps_av)
                        finalize(on_pe=True)
                        feeder.add(o_gen(qc))
                    feeder.drain()

    nc.compile()
    return nc


def host_inputs_causal(x, wq, wk, wv, wo):
    """Build the 8 per-core fp16 input maps for the causal program.

    Layouts match the kernel's SBUF tiles exactly so every DMA is a plain
    2D contiguous transfer:
      xq [NQC*P, NDT*QC]  : xq[qc*P+p, di*QC+j] = x[b, qc*QC+j, di*P+p]
      w*T [P, NDT*E]      : w[p, di*E+e] = W[r][e, di*P+p]     (W is [out,in])
      woT [P, HPC*D]      : wo[p, h*D+o] = WO[o, r][h*P+p]
      aux [P, 2S+JB*QC]   : cosT | sinTs | dmask packed side by side
    """
    S_ = x.shape[1]
    E = HPC * DH
    QCJB = QC * (QC // P)
    f16 = np.float16
    NDT_ = x.shape[2] // P
    NQC_ = S_ // QC
    inv_freq = 1.0 / (10000.0 ** (np.arange(0, DH, 2, dtype=np.float32) / DH))
    t = np.arange(S_, dtype=np.float32)
    freqs = np.outer(t, inv_freq)                      # [S, dh/2]
    emb = np.concatenate([freqs, freqs], axis=-1)      # [S, dh]
    cosT = np.cos(emb).T.astype(f16)
    sinT = np.sin(emb).T.astype(np.float32)
    sinTs = np.concatenate([-sinT[:DH // 2], sinT[DH // 2:]], 0).astype(f16)

    # dmask[p, j*QC + q] = 1 if (j*P + p) <= q else 0  (within a q-chunk)
    j = np.arange(QC // P)[:, None, None]
    pp = np.arange(P)[None, :, None]
    qq = np.arange(QC)[None, None, :]
    dm = (j * P + pp <= qq).astype(f16)                # [JB, P, QC]
    dmask = dm.transpose(1, 0, 2).reshape(P, -1)[:, :P]
    aux = np.ascontiguousarray(
        np.concatenate([cosT, sinTs, dmask], axis=1))  # [P, 2S+P]

    def flat_w(wT):
        # [D, E] -> [P, NDT*E] with col = di*E + e
        return np.ascontiguousarray(
            wT.reshape(NDT_, P, E).transpose(1, 0, 2).reshape(P, NDT_ * E))

    xqs = []
    for b in range(B):
        # [S, D] -> [NQC*P, NDT*QC] with row = qc*P+p, col = di*QC+j
        xb = x[b].astype(f16).reshape(NQC_, QC, NDT_, P)
        xqs.append(np.ascontiguousarray(
            xb.transpose(0, 3, 2, 1).reshape(NQC_ * P, NDT_ * QC)))

    in_maps = []
    for core in range(N_CORES):
        b, g = divmod(core, GROUPS)
        r = slice(g * E, (g + 1) * E)
        woTg = wo[:, r].T.astype(f16)                  # [E, D]
        in_maps.append({
            "xq": xqs[b],
            "wqT": flat_w(wq[r].T.astype(f16)),
            "wkT": flat_w(wk[r].T.astype(f16)),
            "wvT": flat_w(wv[r].T.astype(f16)),
            "woT": np.ascontiguousarray(
                woTg.reshape(HPC, P, D).transpose(1, 0, 2).reshape(P, -1)),
            "aux": aux,
        })
    return in_maps


# ---------------------------------------------------------------------------
# Legacy f32r path (kept for the non-causal mask modes).
# ---------------------------------------------------------------------------

def build_program(S=S, D=D, HPC=HPC, mode="causal"):
    """Legacy per-core Bass/Tile program. mode: "none" | "general"."""
    _ensure_paths()
    import concourse.bass as bass  # noqa: F401
    import concourse.mybir as mybir
    import concourse.tile as tile
    from concourse import bacc

    f32 = mybir.dt.float32
    f32r = mybir.dt.float32r
    Exp = mybir.ActivationFunctionType.Exp

    E = HPC * P          # per-core projection width (512)
    NDT = D // P         # d (contraction) tiles for projections
    NQC = S // QC        # q chunks
    NKT = S // P         # k tiles
    NST = S // P         # s tiles
    NOC = D // QC        # output chunks for O projection
    JB = QC // P         # diagonal blocks per q chunk (4)
    HF = P // 2
    scale = 1.0 / float(np.sqrt(DH))

    nc = bacc.Bacc("TRN2", target_bir_lowering=False, debug=False,
                   num_devices=N_CORES)

    xT = nc.dram_tensor("xT", [D, S], f32r, kind="ExternalInput").ap()
    wqT = nc.dram_tensor("wqT", [D, E], f32r, kind="ExternalInput").ap()
    wkT = nc.dram_tensor("wkT", [D, E], f32r, kind="ExternalInput").ap()
    wvT = nc.dram_tensor("wvT", [D, E], f32r, kind="ExternalInput").ap()
    woT = nc.dram_tensor("woT", [E, D], f32r, kind="ExternalInput").ap()
    cosT = nc.dram_tensor("cosT", [P, S], f32, kind="ExternalInput").ap()
    sinTs = nc.dram_tensor("sinTs", [P, S], f32, kind="ExternalInput").ap()
    if mode == "causal":
        dmask = nc.dram_tensor("dmask", [P, JB * QC], f32,
                               kind="ExternalInput").ap()
    elif mode == "general":
        maskT = nc.dram_tensor("maskT", [S, S], f32, kind="ExternalInput").ap()
    y = nc.dram_tensor("y", [S, D], f32, kind="ExternalOutput").ap()
    qt_s = [nc.dram_tensor(f"qt_s{h}", [P, S], f32r).ap()
            for h in range(HPC)]
    kt_s = [nc.dram_tensor(f"kt_s{h}", [P, S], f32r).ap()
            for h in range(HPC)]

    # DRAM views with d/k tiled onto partitions: [p, tile, col]
    xT_t = xT.rearrange("(t p) s -> p t s", p=P)
    wqT_t = wqT.rearrange("(t p) e -> p t e", p=P)
    wkT_t = wkT.rearrange("(t p) e -> p t e", p=P)
    wvT_t = wvT.rearrange("(t p) e -> p t e", p=P)
    woT_t = woT.rearrange("(t p) o -> p t o", p=P)
    if mode == "general":
        maskT_t = maskT.rearrange("(t p) q -> p t q", p=P)

    def mm(ps, lhsT, rhs, start, stop):
        nc.tensor.matmul(ps, lhsT=lhsT, rhs=rhs, start=start, stop=stop)

    with tile.TileContext(nc) as tc:
      with tc.tile_pool(name="persist", bufs=1) as persist:
        v_sb = persist.tile([P, NST * E], f32r)      # V[k, e]: [p, ki*E + e]
        ones_col = persist.tile([P, 1], f32r)
        ones_row = persist.tile([1, P], f32r)
        qt0_sb = persist.tile([P, S], f32r)
        kt0_sb = persist.tile([P, S], f32r)
        ones_col_f = persist.tile([P, 1], f32)
        ones_row_f = persist.tile([1, P], f32)
        nc.vector.memset(ones_col_f, 1.0)
        nc.vector.memset(ones_row_f, 1.0)
        nc.vector.tensor_copy(ones_col, ones_col_f)
        nc.vector.tensor_copy(ones_row, ones_row_f)

        # ---- Phase A: QKV projections + RoPE (single pass over xT) ----
        with tc.tile_pool(name="wp", bufs=1) as wp, \
             tc.tile_pool(name="csp", bufs=2) as csp, \
             tc.tile_pool(name="xtp", bufs=5) as xtp, \
             tc.tile_pool(name="evp", bufs=2) as evp, \
             tc.tile_pool(name="ps_v", bufs=2, space="PSUM") as ps_v, \
             tc.tile_pool(name="ps_qk", bufs=3, space="PSUM") as ps_qk:
            wv_sb = wp.tile([P, NDT * E], f32r)
            wq_sb = wp.tile([P, NDT * E], f32r)
            wk_sb = wp.tile([P, NDT * E], f32r)
            WG = 4
            def load_w(w_sb, w_t, g, t0=0, tn=None):
                t0 = g * WG + t0
                tn = tn if tn is not None else WG
                nc.sync.dma_start(
                    w_sb[:, t0 * E:(t0 + tn) * E].rearrange(
                        "p (t e) -> p t e", t=tn),
                    w_t[:, t0:t0 + tn])
            load_w(wv_sb, wvT_t, 0, 0, 1)   # 0.5MB: first matmul's dep
            load_w(wv_sb, wvT_t, 0, 1, 3)
            XG = 4                      # d-tiles per xt transfer
            for qc in range(NQC):
                qs = slice(qc * QC, (qc + 1) * QC)
                cos_t = csp.tile([P, QC], f32, tag="cos")
                sin_t = csp.tile([P, QC], f32, tag="sin")
                xg = []
                xts = []
                for g in range(NDT // XG):
                    x_g = xtp.tile([P, XG * QC], f32r, tag="xt")
                    if qc == 0 and g == 0:
                        nc.sync.dma_start(x_g[:, 0:QC], xT_t[:, 0, qs])
                        nc.sync.dma_start(
                            x_g[:, QC:XG * QC].rearrange(
                                "p (t s) -> p t s", t=XG - 1),
                            xT_t[:, 1:XG, qs])
                    else:
                        nc.sync.dma_start(
                            x_g.rearrange("p (t s) -> p t s", t=XG),
                            xT_t[:, g * XG:(g + 1) * XG, qs])
                    xg.append(x_g)
                    if qc == 0 and g < NDT // WG - 1:
                        load_w(wv_sb, wvT_t, g + 1)
                    xts += [x_g[:, j * QC:(j + 1) * QC] for j in range(XG)]
                nc.sync.dma_start(cos_t, cosT[:, qs])
                nc.sync.dma_start(sin_t, sinTs[:, qs])
                if qc == 0:
                    for g in range(NDT // WG):
                        load_w(wq_sb, wqT_t, g)
                        load_w(wk_sb, wkT_t, g)
                # V projection (natural layout)
                for sl in range(QC // P):
                    si = qc * (QC // P) + sl
                    psv = ps_v.tile([P, E], f32, tag="psv")
                    for di in range(NDT):
                        mm(psv, xts[di][:, sl * P:(sl + 1) * P],
                           wv_sb[:, di * E:(di + 1) * E],
                           start=(di == 0), stop=(di == NDT - 1))
                    nc.vector.tensor_copy(v_sb[:, si * E:(si + 1) * E], psv)
                # Q/K projections (transposed layout) + RoPE eviction
                for h in range(HPC):
                    for w_sb, dst, sb0 in ((wq_sb, qt_s, qt0_sb),
                                           (wk_sb, kt_s, kt0_sb)):
                        ps = ps_qk.tile([P, QC], f32, tag="psqk")
                        for di in range(NDT):
                            mm(ps,
                               w_sb[:, di * E + h * P: di * E + (h + 1) * P],
                               xts[di],
                               start=(di == 0), stop=(di == NDT - 1))
                        tmp = evp.tile([P, QC], f32, tag="tmp")
                        nc.vector.tensor_mul(tmp[0:HF, :], ps[HF:P, :],
                                             sin_t[0:HF, :])
                        nc.vector.tensor_mul(tmp[HF:P, :], ps[0:HF, :],
                                             sin_t[HF:P, :])
                        ro2 = evp.tile([P, QC], f32, tag="ro2")
                        nc.vector.tensor_mul(ro2, ps, cos_t)
                        if h == 0:
                            nc.vector.tensor_add(sb0[:, qs], ro2, tmp)
                        else:
                            ro = evp.tile([P, QC], f32r, tag="ro")
                            nc.vector.tensor_add(ro, ro2, tmp)
                            nc.sync.dma_start(dst[h][:, qs], ro)

        # ---- Phases B (attention) and C (O projection) ----
        with tc.tile_pool(name="bcp", bufs=1) as bcp:
            avt = bcp.tile([P, HPC * S], f32r)      # AV^T: [p=e, h*S + q]
            wo_sb = bcp.tile([P, HPC * D], f32r)    # [p=e, h*D + o]
            if mode == "causal":
                mask_sb = bcp.tile([P, JB * QC], f32)
                nc.sync.dma_start(mask_sb, dmask)
            nc.sync.dma_start(
                wo_sb.rearrange("p (t o) -> p t o", t=HPC), woT_t)

            with tc.tile_pool(name="qkp", bufs=2) as qkp, \
                 tc.tile_pool(name="etp", bufs=1) as etp, \
                 tc.tile_pool(name="mkp", bufs=2) as mkp, \
                 tc.tile_pool(name="smp", bufs=2) as smp, \
                 tc.tile_pool(name="ps_sc", bufs=4, space="PSUM") as ps_sc, \
                 tc.tile_pool(name="ps_av", bufs=2, space="PSUM") as ps_av, \
                 tc.tile_pool(name="ps_dn", bufs=1, space="PSUM") as ps_dn, \
                 tc.tile_pool(name="ps_bc", bufs=1, space="PSUM") as ps_bc:
                pending = None

                def finalize(fin):
                    h, qc, ps_avt, den_sb = fin
                    psb = ps_bc.tile([P, QC], f32, tag="bc")
                    mm(psb, ones_row, den_sb, start=True, stop=True)
                    bc_sb = smp.tile([P, QC], f32, tag="bcs")
                    nc.vector.reciprocal_approx_fast(out=bc_sb, in_=psb)
                    nc.vector.tensor_mul(
                        avt[:, h * S + qc * QC: h * S + (qc + 1) * QC],
                        ps_avt, bc_sb)

                for h in range(HPC):
                    if h == 0:
                        qt, kt = qt0_sb, kt0_sb
                    else:
                        qt = qkp.tile([P, S], f32r, tag="qt")
                        kt = qkp.tile([P, S], f32r, tag="kt")
                        nc.sync.dma_start(qt, qt_s[h])
                        nc.sync.dma_start(kt, kt_s[h])
                    for qc in range(NQC):
                        nkt = JB * (qc + 1) if mode == "causal" else NKT
                        qs = slice(qc * QC, (qc + 1) * QC)
                        et = etp.tile([P, NKT * QC], f32r, tag="et")
                        ps_avt = ps_av.tile([P, QC], f32, tag="avt")
                        ps_den = ps_dn.tile([1, QC], f32, tag="den")
                        if mode == "general":
                            mk = mkp.tile([P, NKT * QC], f32, tag="mk")
                            nc.sync.dma_start(
                                mk.rearrange("p (t q) -> p t q", t=NKT),
                                maskT_t[:, :, qs])

                        LAG = 4 if nkt >= 4 else nkt
                        for ki in range(nkt + LAG):
                            if ki < nkt:
                                ps_s = ps_sc.tile([P, QC], f32, tag="sc")
                                mm(ps_s, kt[:, ki * P:(ki + 1) * P], qt[:, qs],
                                   start=True, stop=True)
                                ets = et[:, ki * QC:(ki + 1) * QC]
                                if mode == "general":
                                    nc.vector.tensor_add(
                                        ps_s, ps_s, mk[:, ki * QC:(ki + 1) * QC])
                                nc.scalar.activation(ets, ps_s, Exp, scale=scale)
                                if mode == "causal" and ki >= JB * qc:
                                    j = ki - JB * qc
                                    nc.vector.tensor_mul(
                                        ets, ets,
                                        mask_sb[:, j * QC:(j + 1) * QC])
                            if ki == 5 and pending is not None:
                                finalize(pending)
                                pending = None
                            if ki >= LAG:
                                kj = ki - LAG
                                ets_j = et[:, kj * QC:(kj + 1) * QC]
                                mm(ps_den, ones_col, ets_j,
                                   start=(kj == 0), stop=(kj == nkt - 1))
                                mm(ps_avt,
                                   v_sb[:, kj * E + h * P: kj * E + (h + 1) * P],
                                   ets_j,
                                   start=(kj == 0), stop=(kj == nkt - 1))
                        den_sb = smp.tile([1, QC], f32r, tag="den")
                        nc.vector.tensor_copy(den_sb, ps_den)
                        pending = (h, qc, ps_avt, den_sb)
                if pending is not None:
                    finalize(pending)
                    pending = None

            with tc.tile_pool(name="yp", bufs=2) as yp, \
                 tc.tile_pool(name="ps_c", bufs=4, space="PSUM") as ps_c:
                for si in range(NST):
                    yt = yp.tile([P, D], f32, tag="yt")
                    for oc in range(NOC):
                        psy = ps_c.tile([P, QC], f32, tag="py")
                        for h in range(HPC):
                            mm(psy,
                               avt[:, h * S + si * P: h * S + (si + 1) * P],
                               wo_sb[:, h * D + oc * QC: h * D + (oc + 1) * QC],
                               start=(h == 0), stop=(h == HPC - 1))
                        if si == NST - 1 and oc % 2 == 1:
                            nc.vector.tensor_copy(
                                yt[:, oc * QC:(oc + 1) * QC], psy)
                        else:
                            nc.scalar.copy(yt[:, oc * QC:(oc + 1) * QC], psy)
                        if si == NST - 1:
                            nc.sync.dma_start(
                                y[si * P:(si + 1) * P, oc * QC:(oc + 1) * QC],
                                yt[:, oc * QC:(oc + 1) * QC])
                    if si < NST - 1:
                        nc.sync.dma_start(y[si * P:(si + 1) * P, :], yt)

    nc.compile()
    return nc


def round_f32r(a):
    """Round fp32 to the PE's fp32r (TF32-like, 11-bit mantissa) encoding."""
    u = np.ascontiguousarray(a, dtype=np.float32).view(np.uint32)
    u = ((u.astype(np.int64) + 0x800) & 0xFFFFF000).astype(np.uint32)
    return u.view(np.float32)


def host_inputs(x, attention_mask, wq, wk, wv, wo, mode):
    """Build the 8 per-core input maps from the full problem inputs."""
    if mode == "causal":
        return host_inputs_causal(x, wq, wk, wv, wo)
    S_, D_ = x.shape[1], x.shape[2]
    E = HPC * DH
    inv_freq = 1.0 / (10000.0 ** (np.arange(0, DH, dtype=np.float32)[: DH // 2 * 2:2] / DH))
    inv_freq = 1.0 / (10000.0 ** (np.arange(0, DH, 2, dtype=np.float32) / DH))
    t = np.arange(S_, dtype=np.float32)
    freqs = np.outer(t, inv_freq)                      # [S, dh/2]
    emb = np.concatenate([freqs, freqs], axis=-1)      # [S, dh]
    cosT = np.ascontiguousarray(np.cos(emb).T, dtype=np.float32)
    sinT = np.sin(emb).T.astype(np.float32)
    sinTs = np.concatenate([-sinT[:DH // 2], sinT[DH // 2:]], 0)
    sinTs = np.ascontiguousarray(sinTs, dtype=np.float32)

    extra = {}
    if mode == "general":
        extra["maskT"] = np.ascontiguousarray(
            attention_mask[0, 0].T, dtype=np.float32)

    in_maps = []
    for core in range(N_CORES):
        b, g = divmod(core, GROUPS)
        r = slice(g * E, (g + 1) * E)
        in_maps.append({
            "xT": round_f32r(x[b].T),
            "wqT": round_f32r(wq[r].T),
            "wkT": round_f32r(wk[r].T),
            "wvT": round_f32r(wv[r].T),
            "woT": round_f32r(wo[:, r].T),
            "cosT": cosT,
            "sinTs": sinTs,
            **extra,
        })
    return in_maps


def detect_mode(attention_mask):
    m = attention_mask[0, 0]
    if not np.any(m):
        return "none"
    S_ = m.shape[0]
    causal = np.triu(np.full((S_, S_), -1e9, dtype=np.float32), k=1)
    if np.array_equal(m, causal):
        return "causal"
    return "general"


def kernel(**inputs):
    _ensure_paths()
    from concourse.bass_utils import run_bass_kernel_spmd

    x = np.asarray(inputs["x"], dtype=np.float32)
    mask = np.asarray(inputs["attention_mask"], dtype=np.float32)
    wq = np.asarray(inputs["wq"], dtype=np.float32)
    wk = np.asarray(inputs["wk"], dtype=np.float32)
    wv = np.asarray(inputs["wv"], dtype=np.float32)
    wo = np.asarray(inputs["wo"], dtype=np.float32)

    mode = detect_mode(mask)
    if mode not in _CACHE:
        if mode == "causal":
            _CACHE[mode] = build_program_causal()
        else:
            _CACHE[mode] = build_program(mode=mode)
    nc = _CACHE[mode]

    in_maps = host_inputs(x, mask, wq, wk, wv, wo, mode)
    res = run_bass_kernel_spmd(nc, in_maps, core_ids=list(range(N_CORES)))

    out = np.zeros((B, S, D), dtype=np.float32)
    for core in range(N_CORES):
        b = core // GROUPS
        out[b] += res.results[core]["y"].astype(np.float32)
    return out



# revision 31
# speedup vs baseline: 1.0469x; 1.0469x over previous
"""Trainium2 Bass kernel for a dense causal self-attention block (RoPE + causal
softmax + QKV/O projections).

Sharding: 8 cores = 2 batches x 4 head-groups (tensor parallel over heads).
Each core computes 4 heads of attention for one batch plus the partial output
projection over its heads' dims; the host sums the 4 partial outputs per batch.

Causal-path design (fp16, head-pipelined):
  - All matmul operands are fp16 (PE streams 1 col/cycle, same as f32r, but
    halves DMA and SBUF so everything stays resident).  PSUM accumulation is
    fp32, so precision loss is one rounding per tensor hop.
  - Software pipeline over heads: [V proj || QK(h0)] -> [attn(h) || QK(h+1)]
    for h=0..2 -> [attn(h3) || O proj].  The QK projections (pure PE work)
    fill the PE while ACT/DVE chew on the previous head's softmax, so the
    scalar engine's exp never stalls the PE.
  - Scores are computed transposed: S^T[k, q] = K^T_tile.T @ Q^T.  exp runs on
    ACT straight out of PSUM in merged [128, 1024] instructions with the
    softmax shift folded into the activation bias (exp(s*scale - 5); softmax
    is shift-invariant, and -5 keeps exp sums inside fp16 range: measured
    scores/sqrt(dh) are within [-11, 9]).
  - Softmax denominator: DVE accumulates sum_ki exp-tiles into esum (fp16,
    2x-mode), then ONE GpSimd partition_all_reduce per (h, qc) produces the
    across-k denominator broadcast over partitions.  No PE matmuls for the
    denominator at all (the old design burned 1/3 of attention PE time on
    ones-vector matmuls).
  - Causality: restricted k-tile range + a 0/1 triangle mask multiply on the
    partially-valid [128, (j+1)*128] prefix of the 4 diagonal tiles per block.
  - AV^T = V_tile.T @ E^T lands in the stationary-operand layout the O
    projection wants; normalization (reciprocal_approx_fast + mul) happens at
    the AV PSUM eviction.
"""

import numpy as np

# Problem constants (hardcoded per the harness contract).
B = 2
S = 2048
D = 2048
H = 16
DH = 128
N_CORES = 8
GROUPS = 4          # head-groups (cores per batch)
HPC = H // GROUPS   # heads per core
P = 128             # SBUF partitions
QC = 512            # q/s chunk (f32 PSUM bank = 512 floats)
BIAS = -5.0         # exp(s*scale + BIAS): keeps exp sums in fp16 range
FEED = 4            # feeder matmuls emitted per attention pipeline step
LAGG = 2            # AV trails scores by LAGG groups (of G k-tiles)
G = 2               # k-tiles per exp group (one [128, 1024] ACT instruction)
N_WARM = 16         # dummy PE matmuls to ramp the clock during DMA fill

_CACHE = {}


def _ensure_paths():
    import sys
    for p in ("/opt/trn_rl_repo", "/root/.axon_site/_ro/trn_rl_repo"):
        try:
            import concourse.bass  # noqa: F401
            return
        except Exception:
            if p not in sys.path:
                sys.path.insert(0, p)
    import concourse.bass  # noqa: F401


def build_program_causal(S=S):
    """Head-pipelined fp16 causal program. Returns the compiled Bacc."""
    _ensure_paths()
    import concourse.bass as bass  # noqa: F401
    import concourse.mybir as mybir
    import concourse.tile as tile
    from concourse import bacc
    from concourse import bass_isa

    f32 = mybir.dt.float32
    f16 = mybir.dt.float16
    Exp = mybir.ActivationFunctionType.Exp
    RAdd = bass_isa.ReduceOp.add

    E = HPC * P          # per-core projection width (512)
    NDT = D // P         # d (contraction) tiles for projections
    NQC = S // QC        # q chunks
    NKT = S // P         # k tiles
    NST = S // P         # s tiles
    NOC = D // QC        # output chunks for O projection
    JB = QC // P         # diagonal blocks per q chunk (4)
    HF = P // 2
    scale = 1.0 / float(np.sqrt(DH))

    nc = bacc.Bacc("TRN2", target_bir_lowering=False, debug=False,
                   num_devices=N_CORES)

    # Pre-tiled flat DRAM layouts (built host-side): every DMA below is a
    # plain 2D contiguous-row transfer, so DIRECT2D descriptor issue on the
    # Sync engine stays cheap (the old rearranged 3D APs cost 1-6us each to
    # issue, which alone delayed the first matmul to t=15us).
    xq = nc.dram_tensor("xq", [NQC * P, NDT * QC], f16,
                        kind="ExternalInput").ap()
    wqT = nc.dram_tensor("wqT", [P, NDT * E], f16, kind="ExternalInput").ap()
    wkT = nc.dram_tensor("wkT", [P, NDT * E], f16, kind="ExternalInput").ap()
    wvT = nc.dram_tensor("wvT", [P, NDT * E], f16, kind="ExternalInput").ap()
    woT = nc.dram_tensor("woT", [P, HPC * D], f16, kind="ExternalInput").ap()
    AUXW = 2 * S + JB * QC
    aux = nc.dram_tensor("aux", [P, AUXW], f16, kind="ExternalInput").ap()
    y = nc.dram_tensor("y", [S, D], f16, kind="ExternalOutput").ap()

    xq_v = xq.rearrange("(q p) c -> p q c", p=P)

    def mm(ps, lhsT, rhs, start, stop):
        nc.tensor.matmul(ps, lhsT=lhsT, rhs=rhs, start=start, stop=stop)

    with tile.TileContext(nc) as tc:
      with tc.tile_pool(name="persist", bufs=1) as persist, \
           tc.tile_pool(name="qkp", bufs=2) as qkp, \
           tc.tile_pool(name="evp", bufs=3) as evp, \
           tc.tile_pool(name="smp", bufs=2) as smp:
        v_sb = persist.tile([P, NST * E], f16)     # V[k, e]: [p, ki*E + e]
        avt = persist.tile([P, HPC * S], f16)      # AV^T: [p=e, h*S + q]
        wo_sb = persist.tile([P, HPC * D], f16)    # [p=e, h*D + o]
        aux_sb = persist.tile([P, AUXW], f16)      # cos | sin | mask packed
        cos_sb = aux_sb[:, 0:S]
        sin_sb = aux_sb[:, S:2 * S]
        mask_sb = aux_sb[:, 2 * S:]
        et = persist.tile([P, NKT * QC], f16)      # exp scores for one (h,qc)
        bias_sb = persist.tile([P, 1], mybir.dt.float32)
        nc.vector.memset(bias_sb, BIAS)
        ones_f = persist.tile([P, P], mybir.dt.float32)
        ones16 = persist.tile([P, P], f16)
        nc.vector.memset(ones_f, 1.0)
        nc.vector.tensor_copy(ones16, ones_f)
        # (A PE warmup chain was tried here to pre-ramp the DVFS clock
        # during the DMA fill; the unavoidable data-wait gap right after it
        # resets the ramp, so it bought nothing and was removed.)

        qt_tiles = {}
        kt_tiles = {}

        def rope_evict(ps, dst, qs):
            # dst[:, qs] = ps*cos + shuffle_halves(ps)*sin_signed
            tmp = evp.tile([P, QC], f16, tag="tmp")
            nc.vector.tensor_mul(tmp[0:HF, :], ps[HF:P, :], sin_sb[0:HF, qs])
            nc.vector.tensor_mul(tmp[HF:P, :], ps[0:HF, :], sin_sb[HF:P, qs])
            ro2 = evp.tile([P, QC], f16, tag="ro2")
            nc.vector.tensor_mul(ro2, ps, cos_sb[:, qs])
            # all-f16 add runs in DVE 2x mode (~0.2us); on GpSimd it costs
            # 1.16us and queues up at stage tails, stalling the qt/kt
            # eviction -> ps_qk recycle chain (observed 0.5-1us PE gaps).
            nc.vector.tensor_add(dst[:, qs], ro2, tmp)

        class Feeder:
            def __init__(self):
                self.gens = []

            def add(self, gen):
                self.gens.append(gen)

            def prime(self, gen):
                # advance to the first yield (issues the gen's first DMA
                # now, so the data is in flight before the gen is consumed)
                try:
                    next(gen)
                except StopIteration:
                    return
                self.gens.append(gen)

            def take(self, n):
                c = 0
                while self.gens and c < n:
                    try:
                        kind = next(self.gens[0])
                    except StopIteration:
                        self.gens.pop(0)
                        continue
                    if kind == "mm":
                        c += 1

            def drain(self):
                self.take(1 << 30)

        feeder = Feeder()

        with tc.tile_pool(name="ps_qk", bufs=2, space="PSUM") as ps_qk:

          def qk_block_mms(h, x_t, qc):
              # generator: 2 x (16 matmuls + RoPE eviction) for head h, chunk qc
              qs = slice(qc * QC, (qc + 1) * QC)
              for w_sb, dst in ((wq_sb, qt_tiles[h]), (wk_sb, kt_tiles[h])):
                  ps = ps_qk.tile([P, QC], f32, tag="psqk")
                  for di in range(NDT):
                      mm(ps, w_sb[:, di * E + h * P: di * E + (h + 1) * P],
                         x_t[:, di * QC:(di + 1) * QC],
                         start=(di == 0), stop=(di == NDT - 1))
                      yield "mm"
                  rope_evict(ps, dst, qs)
                  yield "side"

          def qk_gen(h):
              # feeder stream: full QK projection for head h (streams x
              # itself, double-buffered one q-chunk ahead)
              qt_tiles[h] = qkp.tile([P, S], f16, tag="qt", name=f"qt{h}")
              kt_tiles[h] = qkp.tile([P, S], f16, tag="kt", name=f"kt{h}")
              x_cur = xp.tile([P, NDT * QC], f16, tag="x")
              nc.sync.dma_start(x_cur, xq_v[:, 0, :])
              yield "side"
              for qc in range(NQC):
                  if qc + 1 < NQC:
                      x_next = xp.tile([P, NDT * QC], f16, tag="x")
                      nc.sync.dma_start(x_next, xq_v[:, qc + 1, :])
                      yield "side"
                  else:
                      x_next = None
                  yield from qk_block_mms(h, x_cur, qc)
                  x_cur = x_next

          pending = [None]

          def finalize(on_pe=False):
              # den broadcast: partition-sum of esum replicated over
              # partitions, then reciprocal + scale at the AV eviction.
              # Runs one block late so the PE never waits on the DVE esum
              # chain.  Stages 0-3 use a GpSimd partition_all_reduce (frees
              # 512 PE cycles per block); stage 4 keeps the PE ones-matmul
              # because its result feeds the O projection immediately.
              # (GpSimd partition_all_reduce was tried here: at 3.5us per
              # [128,512] reduce it stalls the ps_av PSUM recycle chain and
              # costs ~100us end to end.  The PE ones-matmul stays.)
              if pending[0] is None:
                  return
              fh, fqc, f_avt, f_esum = pending[0]
              pending[0] = None
              psd = ps_qk.tile([P, QC], f32, tag="psqk")
              mm(psd, ones16, f_esum, start=True, stop=True)
              rec = smp.tile([P, QC], f32, tag="rec", bufs=1)
              nc.vector.reciprocal_approx_fast(out=rec, in_=psd)
              nc.vector.tensor_mul(
                  avt[:, fh * S + fqc * QC: fh * S + (fqc + 1) * QC],
                  f_avt, rec)

          def attn_block(h, qc, ps_sc, ps_av):
              nkt = JB * (qc + 1)
              ngr = nkt // G
              qs = slice(qc * QC, (qc + 1) * QC)
              qt = qt_tiles[h]
              kt = kt_tiles[h]
              ps_avt = ps_av.tile([P, QC], f32, tag="avt")
              esum = smp.tile([P, QC], f16, tag="esum")

              def cstart(ki):
                  # first causally-valid col (within the q-chunk) of k-tile ki
                  j = ki - JB * qc
                  return j * P if j > 0 else 0

              def av_group(g):
                  for t in range(G):
                      kj = g * G + t
                      cs = cstart(kj)
                      mm(ps_avt[:, cs:],
                         v_sb[:, kj * E + h * P: kj * E + (h + 1) * P],
                         et[:, kj * QC + cs:(kj + 1) * QC],
                         start=(kj == 0), stop=(kj == nkt - 1))

              for g in range(ngr + LAGG):
                  if g < ngr:
                      ps_s = ps_sc.tile([P, G * QC], f32, tag="sc")
                      css = [cstart(g * G + t) for t in range(G)]
                      for t in range(G):
                          ki = g * G + t
                          mm(ps_s[:, t * QC + css[t]:(t + 1) * QC],
                             kt[:, ki * P:(ki + 1) * P],
                             qt[:, qc * QC + css[t]:(qc + 1) * QC],
                             start=True, stop=True)
                  if g == min(1, ngr - 1):
                      finalize()
                  feeder.take(FEED)
                  if g < ngr:
                      if not any(css):
                          nc.scalar.activation(
                              et[:, g * G * QC:(g + 1) * G * QC], ps_s, Exp,
                              scale=scale, bias=bias_sb)
                      else:
                          for t in range(G):
                              ki = g * G + t
                              nc.scalar.activation(
                                  et[:, ki * QC + css[t]:(ki + 1) * QC],
                                  ps_s[:, t * QC + css[t]:(t + 1) * QC],
                                  Exp, scale=scale, bias=bias_sb)
                      for t in range(G):
                          ki = g * G + t
                          if ki - JB * qc >= 0:
                              cs = cstart(ki)
                              # triangle block only: cols [cs, cs+P)
                              nc.vector.tensor_mul(
                                  et[:, ki * QC + cs: ki * QC + cs + P],
                                  et[:, ki * QC + cs: ki * QC + cs + P],
                                  mask_sb[:, 0:P])
                      for t in range(G):
                          ki = g * G + t
                          cs = css[t]
                          ets = et[:, ki * QC + cs:(ki + 1) * QC]
                          if ki == 0:
                              nc.vector.tensor_copy(esum, ets)
                          else:
                              nc.vector.tensor_add(
                                  esum[:, cs:], esum[:, cs:], ets)
                  if g >= LAGG:
                      av_group(g - LAGG)
              pending[0] = (h, qc, ps_avt, esum)

          # One x pool shared by stage 0 and the per-head QK feeder streams:
          # a fresh pool would be allocated in the freed SBUF region of the
          # old one, and the region-based WAR tracking would then stall the
          # feeder's first x DMA until every stage-0 read of that region
          # finished (a ~30us stall observed on hardware).  Sharing the pool
          # (and tile tag) gives seamless buffer rotation instead.
          with tc.tile_pool(name="wp", bufs=1) as wp, \
               tc.tile_pool(name="xp", bufs=3) as xp:
            wv_sb = wp.tile([P, NDT * E], f16)
            wq_sb = wp.tile([P, NDT * E], f16)
            wk_sb = wp.tile([P, NDT * E], f16)

            WG = 4

            def load_w(w_sb, w_t, g):
                nc.sync.dma_start(w_sb[:, g * WG * E:(g + 1) * WG * E],
                                  w_t[:, g * WG * E:(g + 1) * WG * E])

            # ---- stage 0: V projection || QK(h0), DMA-paced startup ----
            with tc.tile_pool(name="ps_v", bufs=4, space="PSUM") as ps_v:
                xs0 = []
                for qc in range(NQC):
                    xs0.append(xp.tile([P, NDT * QC], f16, tag="x", name=f"x0_{qc}"))
                # priming DMAs in first-use order; wv and x(qc0) interleaved
                # in di-chunks so the di-outer V loop streams at DMA pace.
                # First pair is a single di-tile so the first matmul's gate
                # is only ~256KB of transfer.
                nc.sync.dma_start(wv_sb[:, 0:E], wvT[:, 0:E])
                nc.scalar.dma_start(xs0[0][:, 0:QC], xq_v[:, 0, 0:QC])
                nc.sync.dma_start(wv_sb[:, E:WG * E], wvT[:, E:WG * E])
                nc.scalar.dma_start(xs0[0][:, QC:WG * QC],
                                    xq_v[:, 0, QC:WG * QC])
                for g in range(1, NDT // WG):
                    load_w(wv_sb, wvT, g)
                    nc.scalar.dma_start(
                        xs0[0][:, g * WG * QC:(g + 1) * WG * QC],
                        xq_v[:, 0, g * WG * QC:(g + 1) * WG * QC])
                # wq interleaved with x1 (and wk with x2) at tile-group
                # granularity: QK(h0,qc0) needs wq at ~29us and V(qc1)
                # consumes x1 progressively — serializing either behind the
                # other's full 2MB stalls its consumer by ~2us.
                for g in range(NDT // WG):
                    load_w(wq_sb, wqT, g)
                    if NQC > 1:
                        nc.scalar.dma_start(
                            xs0[1][:, g * WG * QC:(g + 1) * WG * QC],
                            xq_v[:, 1, g * WG * QC:(g + 1) * WG * QC])
                for g in range(NDT // WG):
                    load_w(wk_sb, wkT, g)
                    if NQC > 2:
                        nc.scalar.dma_start(
                            xs0[2][:, g * WG * QC:(g + 1) * WG * QC],
                            xq_v[:, 2, g * WG * QC:(g + 1) * WG * QC])
                nc.sync.dma_start(aux_sb, aux)
                nc.sync.dma_start(wo_sb, woT)
                for qc in range(3, NQC):
                    nc.scalar.dma_start(xs0[qc], xq_v[:, qc, :])

                qt_tiles[0] = qkp.tile([P, S], f16, tag="qt", name="qt0")
                kt_tiles[0] = qkp.tile([P, S], f16, tag="kt", name="kt0")

                def v_block(qc):
                    # di-outer so the first matmul only needs the first
                    # wv/x di-chunk; evictions on ACT (idle during stage 0).
                    psv = [ps_v.tile([P, E], f32, tag="psv", name=f"psv{sl}")
                           for sl in range(QC // P)]
                    for di in range(NDT):
                        for sl in range(QC // P):
                            mm(psv[sl],
                               xs0[qc][:, di * QC + sl * P: di * QC + (sl + 1) * P],
                               wv_sb[:, di * E:(di + 1) * E],
                               start=(di == 0), stop=(di == NDT - 1))
                    for sl in range(QC // P):
                        si = qc * (QC // P) + sl
                        if qc == NQC - 1:
                            nc.vector.tensor_copy(
                                v_sb[:, si * E:(si + 1) * E], psv[sl])
                        else:
                            nc.scalar.copy(v_sb[:, si * E:(si + 1) * E], psv[sl])

                def qk0_block(qc):
                    for _ in qk_block_mms(0, xs0[qc], qc):
                        pass

                # emission order tuned so x(qc) WAR frees line up with the
                # DMA issue order above: V0 V1 QK0 V2 QK1 V3 QK2 QK3
                order = []
                for qc in range(NQC):
                    order.append(("v", qc))
                    if qc >= 1 or NQC == 1:
                        order.append(("qk", qc - 1 if NQC > 1 else 0))
                if NQC > 1:
                    order += [("qk", NQC - 2), ("qk", NQC - 1)]
                seen = set()
                for kind, qc in order:
                    if (kind, qc) in seen:
                        continue
                    seen.add((kind, qc))
                    if kind == "v":
                        v_block(qc)
                    else:
                        qk0_block(qc)

            # ---- stages 1..3: attn(h) || QK(h+1) ----
            # Each head's QK gen is primed one stage early so its first x
            # slab is in flight before its matmuls are pulled; no drains at
            # stage boundaries (the ~16-matmul backlog of head h+1 drains
            # during head h+1's early feed steps, which only need the
            # earliest q-chunks of qt/kt).
            with tc.tile_pool(name="ps_sc", bufs=2, space="PSUM") as ps_sc, \
                 tc.tile_pool(name="ps_av", bufs=2, space="PSUM") as ps_av:
                feeder.prime(qk_gen(1))
                for h in range(HPC - 1):
                    for qc in range(NQC):
                        attn_block(h, qc, ps_sc, ps_av)
                    if h + 2 < HPC:
                        feeder.prime(qk_gen(h + 2))
                finalize()

                # ---- stage 4: attn(h3) || O projection ----
                # Shares ps_sc/ps_av: a fresh PSUM pool would reuse their
                # banks, and the region-based tracking would then stall
                # stage 4 behind the full stage-3 PSUM drain (~1us gap).
                with tc.tile_pool(name="ytp", bufs=2) as ytp:

                    def o_gen(qc):
                        for si in range(qc * (QC // P), (qc + 1) * (QC // P)):
                            yt = ytp.tile([P, D], f16, tag="yt")
                            last = si == NST - 1
                            for oc in range(NOC):
                                psy = ps_qk.tile([P, QC], f32, tag="psqk")
                                for h in range(HPC):
                                    mm(psy,
                                       avt[:, h * S + si * P:
                                           h * S + (si + 1) * P],
                                       wo_sb[:, h * D + oc * QC:
                                             h * D + (oc + 1) * QC],
                                       start=(h == 0), stop=(h == HPC - 1))
                                    yield "mm"
                                nc.scalar.copy(
                                    yt[:, oc * QC:(oc + 1) * QC], psy)
                                yield "side"
                                if last:
                                    # per-oc writes so the final transfer on
                                    # the critical tail is 128KB, not 512KB
                                    nc.sync.dma_start(
                                        y[si * P:(si + 1) * P,
                                          oc * QC:(oc + 1) * QC],
                                        yt[:, oc * QC:(oc + 1) * QC])
                                    yield "side"
                            if not last:
                                nc.sync.dma_start(
                                    y[si * P:(si + 1) * P, :], yt)
                                yield "side"

                    for qc in range(NQC):
                        attn_block(HPC - 1, qc, ps_sc, ps_av)
                        finalize(on_pe=True)
                        feeder.add(o_gen(qc))
                    feeder.drain()

    nc.compile()
    return nc


def host_inputs_causal(x, wq, wk, wv, wo):
    """Build the 8 per-core fp16 input maps for the causal program.

    Layouts match the kernel's SBUF tiles exactly so every DMA is a plain
    2D contiguous transfer:
      xq [NQC*P, NDT*QC]  : xq[qc*P+p, di*QC+j] = x[b, qc*QC+j, di*P+p]
      w*T [P, NDT*E]      : w[p, di*E+e] = W[r][e, di*P+p]     (W is [out,in])
      woT [P, HPC*D]      : wo[p, h*D+o] = WO[o, r][h*P+p]
      aux [P, 2S+JB*QC]   : cosT | sinTs | dmask packed side by side
    """
    S_ = x.shape[1]
    E = HPC * DH
    QCJB = QC * (QC // P)
    f16 = np.float16
    NDT_ = x.shape[2] // P
    NQC_ = S_ // QC
    inv_freq = 1.0 / (10000.0 ** (np.arange(0, DH, 2, dtype=np.float32) / DH))
    t = np.arange(S_, dtype=np.float32)
    freqs = np.outer(t, inv_freq)                      # [S, dh/2]
    emb = np.concatenate([freqs, freqs], axis=-1)      # [S, dh]
    cosT = np.cos(emb).T.astype(f16)
    sinT = np.sin(emb).T.astype(np.float32)
    sinTs = np.concatenate([-sinT[:DH // 2], sinT[DH // 2:]], 0).astype(f16)

    # dmask[p, j*QC + q] = 1 if (j*P + p) <= q else 0  (within a q-chunk)
    j = np.arange(QC // P)[:, None, None]
    pp = np.arange(P)[None, :, None]
    qq = np.arange(QC)[None, None, :]
    dm = (j * P + pp <= qq).astype(f16)                # [JB, P, QC]
    dmask = dm.transpose(1, 0, 2).reshape(P, -1)
    aux = np.ascontiguousarray(
        np.concatenate([cosT, sinTs, dmask], axis=1))  # [P, 2S+JB*QC]

    def flat_w(wT):
        # [D, E] -> [P, NDT*E] with col = di*E + e
        return np.ascontiguousarray(
            wT.reshape(NDT_, P, E).transpose(1, 0, 2).reshape(P, NDT_ * E))

    xqs = []
    for b in range(B):
        # [S, D] -> [NQC*P, NDT*QC] with row = qc*P+p, col = di*QC+j
        xb = x[b].astype(f16).reshape(NQC_, QC, NDT_, P)
        xqs.append(np.ascontiguousarray(
            xb.transpose(0, 3, 2, 1).reshape(NQC_ * P, NDT_ * QC)))

    in_maps = []
    for core in range(N_CORES):
        b, g = divmod(core, GROUPS)
        r = slice(g * E, (g + 1) * E)
        woTg = wo[:, r].T.astype(f16)                  # [E, D]
        in_maps.append({
            "xq": xqs[b],
            "wqT": flat_w(wq[r].T.astype(f16)),
            "wkT": flat_w(wk[r].T.astype(f16)),
            "wvT": flat_w(wv[r].T.astype(f16)),
            "woT": np.ascontiguousarray(
                woTg.reshape(HPC, P, D).transpose(1, 0, 2).reshape(P, -1)),
            "aux": aux,
        })
    return in_maps


# ---------------------------------------------------------------------------
# Legacy f32r path (kept for the non-causal mask modes).
# ---------------------------------------------------------------------------

def build_program(S=S, D=D, HPC=HPC, mode="causal"):
    """Legacy per-core Bass/Tile program. mode: "none" | "general"."""
    _ensure_paths()
    import concourse.bass as bass  # noqa: F401
    import concourse.mybir as mybir
    import concourse.tile as tile
    from concourse import bacc

    f32 = mybir.dt.float32
    f32r = mybir.dt.float32r
    Exp = mybir.ActivationFunctionType.Exp

    E = HPC * P          # per-core projection width (512)
    NDT = D // P         # d (contraction) tiles for projections
    NQC = S // QC        # q chunks
    NKT = S // P         # k tiles
    NST = S // P         # s tiles
    NOC = D // QC        # output chunks for O projection
    JB = QC // P         # diagonal blocks per q chunk (4)
    HF = P // 2
    scale = 1.0 / float(np.sqrt(DH))

    nc = bacc.Bacc("TRN2", target_bir_lowering=False, debug=False,
                   num_devices=N_CORES)

    xT = nc.dram_tensor("xT", [D, S], f32r, kind="ExternalInput").ap()
    wqT = nc.dram_tensor("wqT", [D, E], f32r, kind="ExternalInput").ap()
    wkT = nc.dram_tensor("wkT", [D, E], f32r, kind="ExternalInput").ap()
    wvT = nc.dram_tensor("wvT", [D, E], f32r, kind="ExternalInput").ap()
    woT = nc.dram_tensor("woT", [E, D], f32r, kind="ExternalInput").ap()
    cosT = nc.dram_tensor("cosT", [P, S], f32, kind="ExternalInput").ap()
    sinTs = nc.dram_tensor("sinTs", [P, S], f32, kind="ExternalInput").ap()
    if mode == "causal":
        dmask = nc.dram_tensor("dmask", [P, JB * QC], f32,
                               kind="ExternalInput").ap()
    elif mode == "general":
        maskT = nc.dram_tensor("maskT", [S, S], f32, kind="ExternalInput").ap()
    y = nc.dram_tensor("y", [S, D], f32, kind="ExternalOutput").ap()
    qt_s = [nc.dram_tensor(f"qt_s{h}", [P, S], f32r).ap()
            for h in range(HPC)]
    kt_s = [nc.dram_tensor(f"kt_s{h}", [P, S], f32r).ap()
            for h in range(HPC)]

    # DRAM views with d/k tiled onto partitions: [p, tile, col]
    xT_t = xT.rearrange("(t p) s -> p t s", p=P)
    wqT_t = wqT.rearrange("(t p) e -> p t e", p=P)
    wkT_t = wkT.rearrange("(t p) e -> p t e", p=P)
    wvT_t = wvT.rearrange("(t p) e -> p t e", p=P)
    woT_t = woT.rearrange("(t p) o -> p t o", p=P)
    if mode == "general":
        maskT_t = maskT.rearrange("(t p) q -> p t q", p=P)

    def mm(ps, lhsT, rhs, start, stop):
        nc.tensor.matmul(ps, lhsT=lhsT, rhs=rhs, start=start, stop=stop)

    with tile.TileContext(nc) as tc:
      with tc.tile_pool(name="persist", bufs=1) as persist:
        v_sb = persist.tile([P, NST * E], f32r)      # V[k, e]: [p, ki*E + e]
        ones_col = persist.tile([P, 1], f32r)
        ones_row = persist.tile([1, P], f32r)
        qt0_sb = persist.tile([P, S], f32r)
        kt0_sb = persist.tile([P, S], f32r)
        ones_col_f = persist.tile([P, 1], f32)
        ones_row_f = persist.tile([1, P], f32)
        nc.vector.memset(ones_col_f, 1.0)
        nc.vector.memset(ones_row_f, 1.0)
        nc.vector.tensor_copy(ones_col, ones_col_f)
        nc.vector.tensor_copy(ones_row, ones_row_f)

        # ---- Phase A: QKV projections + RoPE (single pass over xT) ----
        with tc.tile_pool(name="wp", bufs=1) as wp, \
             tc.tile_pool(name="csp", bufs=2) as csp, \
             tc.tile_pool(name="xtp", bufs=5) as xtp, \
             tc.tile_pool(name="evp", bufs=2) as evp, \
             tc.tile_pool(name="ps_v", bufs=2, space="PSUM") as ps_v, \
             tc.tile_pool(name="ps_qk", bufs=3, space="PSUM") as ps_qk:
            wv_sb = wp.tile([P, NDT * E], f32r)
            wq_sb = wp.tile([P, NDT * E], f32r)
            wk_sb = wp.tile([P, NDT * E], f32r)
            WG = 4
            def load_w(w_sb, w_t, g, t0=0, tn=None):
                t0 = g * WG + t0
                tn = tn if tn is not None else WG
                nc.sync.dma_start(
                    w_sb[:, t0 * E:(t0 + tn) * E].rearrange(
                        "p (t e) -> p t e", t=tn),
                    w_t[:, t0:t0 + tn])
            load_w(wv_sb, wvT_t, 0, 0, 1)   # 0.5MB: first matmul's dep
            load_w(wv_sb, wvT_t, 0, 1, 3)
            XG = 4                      # d-tiles per xt transfer
            for qc in range(NQC):
                qs = slice(qc * QC, (qc + 1) * QC)
                cos_t = csp.tile([P, QC], f32, tag="cos")
                sin_t = csp.tile([P, QC], f32, tag="sin")
                xg = []
                xts = []
                for g in range(NDT // XG):
                    x_g = xtp.tile([P, XG * QC], f32r, tag="xt")
                    if qc == 0 and g == 0:
                        nc.sync.dma_start(x_g[:, 0:QC], xT_t[:, 0, qs])
                        nc.sync.dma_start(
                            x_g[:, QC:XG * QC].rearrange(
                                "p (t s) -> p t s", t=XG - 1),
                            xT_t[:, 1:XG, qs])
                    else:
                        nc.sync.dma_start(
                            x_g.rearrange("p (t s) -> p t s", t=XG),
                            xT_t[:, g * XG:(g + 1) * XG, qs])
                    xg.append(x_g)
                    if qc == 0 and g < NDT // WG - 1:
                        load_w(wv_sb, wvT_t, g + 1)
                    xts += [x_g[:, j * QC:(j + 1) * QC] for j in range(XG)]
                nc.sync.dma_start(cos_t, cosT[:, qs])
                nc.sync.dma_start(sin_t, sinTs[:, qs])
                if qc == 0:
                    for g in range(NDT // WG):
                        load_w(wq_sb, wqT_t, g)
                        load_w(wk_sb, wkT_t, g)
                # V projection (natural layout)
                for sl in range(QC // P):
                    si = qc * (QC // P) + sl
                    psv = ps_v.tile([P, E], f32, tag="psv")
                    for di in range(NDT):
                        mm(psv, xts[di][:, sl * P:(sl + 1) * P],
                           wv_sb[:, di * E:(di + 1) * E],
                           start=(di == 0), stop=(di == NDT - 1))
                    nc.vector.tensor_copy(v_sb[:, si * E:(si + 1) * E], psv)
                # Q/K projections (transposed layout) + RoPE eviction
                for h in range(HPC):
                    for w_sb, dst, sb0 in ((wq_sb, qt_s, qt0_sb),
                                           (wk_sb, kt_s, kt0_sb)):
                        ps = ps_qk.tile([P, QC], f32, tag="psqk")
                        for di in range(NDT):
                            mm(ps,
                               w_sb[:, di * E + h * P: di * E + (h + 1) * P],
                               xts[di],
                               start=(di == 0), stop=(di == NDT - 1))
                        tmp = evp.tile([P, QC], f32, tag="tmp")
                        nc.vector.tensor_mul(tmp[0:HF, :], ps[HF:P, :],
                                             sin_t[0:HF, :])
                        nc.vector.tensor_mul(tmp[HF:P, :], ps[0:HF, :],
                                             sin_t[HF:P, :])
                        ro2 = evp.tile([P, QC], f32, tag="ro2")
                        nc.vector.tensor_mul(ro2, ps, cos_t)
                        if h == 0:
                            nc.vector.tensor_add(sb0[:, qs], ro2, tmp)
                        else:
                            ro = evp.tile([P, QC], f32r, tag="ro")
                            nc.vector.tensor_add(ro, ro2, tmp)
                            nc.sync.dma_start(dst[h][:, qs], ro)

        # ---- Phases B (attention) and C (O projection) ----
        with tc.tile_pool(name="bcp", bufs=1) as bcp:
            avt = bcp.tile([P, HPC * S], f32r)      # AV^T: [p=e, h*S + q]
            wo_sb = bcp.tile([P, HPC * D], f32r)    # [p=e, h*D + o]
            if mode == "causal":
                mask_sb = bcp.tile([P, JB * QC], f32)
                nc.sync.dma_start(mask_sb, dmask)
            nc.sync.dma_start(
                wo_sb.rearrange("p (t o) -> p t o", t=HPC), woT_t)

            with tc.tile_pool(name="qkp", bufs=2) as qkp, \
                 tc.tile_pool(name="etp", bufs=1) as etp, \
                 tc.tile_pool(name="mkp", bufs=2) as mkp, \
                 tc.tile_pool(name="smp", bufs=2) as smp, \
                 tc.tile_pool(name="ps_sc", bufs=4, space="PSUM") as ps_sc, \
                 tc.tile_pool(name="ps_av", bufs=2, space="PSUM") as ps_av, \
                 tc.tile_pool(name="ps_dn", bufs=1, space="PSUM") as ps_dn, \
                 tc.tile_pool(name="ps_bc", bufs=1, space="PSUM") as ps_bc:
                pending = None

                def finalize(fin):
                    h, qc, ps_avt, den_sb = fin
                    psb = ps_bc.tile([P, QC], f32, tag="bc")
                    mm(psb, ones_row, den_sb, start=True, stop=True)
                    bc_sb = smp.tile([P, QC], f32, tag="bcs")
                    nc.vector.reciprocal_approx_fast(out=bc_sb, in_=psb)
                    nc.vector.tensor_mul(
                        avt[:, h * S + qc * QC: h * S + (qc + 1) * QC],
                        ps_avt, bc_sb)

                for h in range(HPC):
                    if h == 0:
                        qt, kt = qt0_sb, kt0_sb
                    else:
                        qt = qkp.tile([P, S], f32r, tag="qt")
                        kt = qkp.tile([P, S], f32r, tag="kt")
                        nc.sync.dma_start(qt, qt_s[h])
                        nc.sync.dma_start(kt, kt_s[h])
                    for qc in range(NQC):
                        nkt = JB * (qc + 1) if mode == "causal" else NKT
                        qs = slice(qc * QC, (qc + 1) * QC)
                        et = etp.tile([P, NKT * QC], f32r, tag="et")
                        ps_avt = ps_av.tile([P, QC], f32, tag="avt")
                        ps_den = ps_dn.tile([1, QC], f32, tag="den")
                        if mode == "general":
                            mk = mkp.tile([P, NKT * QC], f32, tag="mk")
                            nc.sync.dma_start(
                                mk.rearrange("p (t q) -> p t q", t=NKT),
                                maskT_t[:, :, qs])

                        LAG = 4 if nkt >= 4 else nkt
                        for ki in range(nkt + LAG):
                            if ki < nkt:
                                ps_s = ps_sc.tile([P, QC], f32, tag="sc")
                                mm(ps_s, kt[:, ki * P:(ki + 1) * P], qt[:, qs],
                                   start=True, stop=True)
                                ets = et[:, ki * QC:(ki + 1) * QC]
                                if mode == "general":
                                    nc.vector.tensor_add(
                                        ps_s, ps_s, mk[:, ki * QC:(ki + 1) * QC])
                                nc.scalar.activation(ets, ps_s, Exp, scale=scale)
                                if mode == "causal" and ki >= JB * qc:
                                    j = ki - JB * qc
                                    nc.vector.tensor_mul(
                                        ets, ets,
                                        mask_sb[:, j * QC:(j + 1) * QC])
                            if ki == 5 and pending is not None:
                                finalize(pending)
                                pending = None
                            if ki >= LAG:
                                kj = ki - LAG
                                ets_j = et[:, kj * QC:(kj + 1) * QC]
                                mm(ps_den, ones_col, ets_j,
                                   start=(kj == 0), stop=(kj == nkt - 1))
                                mm(ps_avt,
                                   v_sb[:, kj * E + h * P: kj * E + (h + 1) * P],
                                   ets_j,
                                   start=(kj == 0), stop=(kj == nkt - 1))
                        den_sb = smp.tile([1, QC], f32r, tag="den")
                        nc.vector.tensor_copy(den_sb, ps_den)
                        pending = (h, qc, ps_avt, den_sb)
                if pending is not None:
                    finalize(pending)
                    pending = None

            with tc.tile_pool(name="yp", bufs=2) as yp, \
                 tc.tile_pool(name="ps_c", bufs=4, space="PSUM") as ps_c:
                for si in range(NST):
                    yt = yp.tile([P, D], f32, tag="yt")
                    for oc in range(NOC):
                        psy = ps_c.tile([P, QC], f32, tag="py")
                        for h in range(HPC):
                            mm(psy,
                               avt[:, h * S + si * P: h * S + (si + 1) * P],
                               wo_sb[:, h * D + oc * QC: h * D + (oc + 1) * QC],
                               start=(h == 0), stop=(h == HPC - 1))
                        if si == NST - 1 and oc % 2 == 1:
                            nc.vector.tensor_copy(
                                yt[:, oc * QC:(oc + 1) * QC], psy)
                        else:
                            nc.scalar.copy(yt[:, oc * QC:(oc + 1) * QC], psy)
                        if si == NST - 1:
                            nc.sync.dma_start(
                                y[si * P:(si + 1) * P, oc * QC:(oc + 1) * QC],
                                yt[:, oc * QC:(oc + 1) * QC])
                    if si < NST - 1:
                        nc.sync.dma_start(y[si * P:(si + 1) * P, :], yt)

    nc.compile()
    return nc


def round_f32r(a):
    """Round fp32 to the PE's fp32r (TF32-like, 11-bit mantissa) encoding."""
    u = np.ascontiguousarray(a, dtype=np.float32).view(np.uint32)
    u = ((u.astype(np.int64) + 0x800) & 0xFFFFF000).astype(np.uint32)
    return u.view(np.float32)


def host_inputs(x, attention_mask, wq, wk, wv, wo, mode):
    """Build the 8 per-core input maps from the full problem inputs."""
    if mode == "causal":
        return host_inputs_causal(x, wq, wk, wv, wo)
    S_, D_ = x.shape[1], x.shape[2]
    E = HPC * DH
    inv_freq = 1.0 / (10000.0 ** (np.arange(0, DH, dtype=np.float32)[: DH // 2 * 2:2] / DH))
    inv_freq = 1.0 / (10000.0 ** (np.arange(0, DH, 2, dtype=np.float32) / DH))
    t = np.arange(S_, dtype=np.float32)
    freqs = np.outer(t, inv_freq)                      # [S, dh/2]
    emb = np.concatenate([freqs, freqs], axis=-1)      # [S, dh]
    cosT = np.ascontiguousarray(np.cos(emb).T, dtype=np.float32)
    sinT = np.sin(emb).T.astype(np.float32)
    sinTs = np.concatenate([-sinT[:DH // 2], sinT[DH // 2:]], 0)
    sinTs = np.ascontiguousarray(sinTs, dtype=np.float32)

    extra = {}
    if mode == "general":
        extra["maskT"] = np.ascontiguousarray(
            attention_mask[0, 0].T, dtype=np.float32)

    in_maps = []
    for core in range(N_CORES):
        b, g = divmod(core, GROUPS)
        r = slice(g * E, (g + 1) * E)
        in_maps.append({
            "xT": round_f32r(x[b].T),
            "wqT": round_f32r(wq[r].T),
            "wkT": round_f32r(wk[r].T),
            "wvT": round_f32r(wv[r].T),
            "woT": round_f32r(wo[:, r].T),
            "cosT": cosT,
            "sinTs": sinTs,
            **extra,
        })
    return in_maps


def detect_mode(attention_mask):
    m = attention_mask[0, 0]
    if not np.any(m):
        return "none"
    S_ = m.shape[0]
    causal = np.triu(np.full((S_, S_), -1e9, dtype=np.float32), k=1)
    if np.array_equal(m, causal):
        return "causal"
    return "general"


def kernel(**inputs):
    _ensure_paths()
    from concourse.bass_utils import run_bass_kernel_spmd

    x = np.asarray(inputs["x"], dtype=np.float32)
    mask = np.asarray(inputs["attention_mask"], dtype=np.float32)
    wq = np.asarray(inputs["wq"], dtype=np.float32)
    wk = np.asarray(inputs["wk"], dtype=np.float32)
    wv = np.asarray(inputs["wv"], dtype=np.float32)
    wo = np.asarray(inputs["wo"], dtype=np.float32)

    mode = detect_mode(mask)
    if mode not in _CACHE:
        if mode == "causal":
            _CACHE[mode] = build_program_causal()
        else:
            _CACHE[mode] = build_program(mode=mode)
    nc = _CACHE[mode]

    in_maps = host_inputs(x, mask, wq, wk, wv, wo, mode)
    res = run_bass_kernel_spmd(nc, in_maps, core_ids=list(range(N_CORES)))

    out = np.zeros((B, S, D), dtype=np.float32)
    for core in range(N_CORES):
        b = core // GROUPS
        out[b] += res.results[core]["y"].astype(np.float32)
    return out

